# revision 43
# baseline (speedup 1.0000x reference)
"""Trainium2 Bass kernel for nn_Estor_concat (scatter_memory).

Math (exact reformulation of the reference):
  v_tag = (tag_emb @ Wv.T + bv) @ out_proj_w.T + out_proj_b            [T, H]
  W_eff[t, j] = sum_h v_tag[t, h] * ff1_w[j, t*H + h]                  [T, H]
  counts[t, b, s] = #spans(tag=t, batch=b) covering s
                  = sum_n onehot_t[n] * ((s < end_n) - (s < start_n))   (PE matmul)
  h1 = counts_b.T @ W_eff + ff1_b ; h2 = relu(h1) @ ff2_w.T + ff2_b
  x = [word_emb_b | h2]; LayerNorm folded into the output projection:
  out = (x @ (lin_w.T * g) - mu * c1) * rstd + (lin_w @ b + lin_b)

Sharding: data-parallel over batch (8 cores, 1 batch each); the W_eff
computation is sharded over tags (2 tags/core) with one AllGather. The
schedule front-loads the W_eff chain so the AllGather (~15us launch
latency) overlaps counts, the word-embedding half of the output/stats
accumulation, and all remaining loads.
"""

import ml_dtypes
import numpy as np

import concourse.bacc as bacc
import concourse.bass as bass
import concourse.mybir as mybir
import concourse.tile as tile
from concourse.bass_utils import run_bass_kernel_spmd

T, B, S, H = 16, 8, 512, 768
H2 = 384
NEW_H = H + H2          # 1152
NL = 33                 # num labels
EPS = 1e-12
NCORES = 8
TPC = T // NCORES       # tags per core = 2
KC_H = H // 128         # 6 chunks of the hidden dim
KC_H2 = H2 // 128       # 3
KC_F = NEW_H // 128     # 9
P = 128
HH = H // 2             # 384 (psum-bank-sized half of H)
ML = 65                 # raw-matmul lhsT cols: [sum | 31 pad | 33 labels]

F32 = mybir.dt.float32
BF16 = mybir.dt.bfloat16
F16 = mybir.dt.float16


def build_kernel(n_span_tiles: int):
    nc = bacc.Bacc(
        "TRN2",
        target_bir_lowering=False,
        debug=False,
        enable_asserts=True,
        num_devices=NCORES,
    )

    def inp(name, shape, dtype=F32):
        return nc.dram_tensor(name, list(shape), dtype, kind="ExternalInput").ap()

    # per-core inputs (host pre-sharded / pre-transposed / pre-chunked)
    we_t = inp("we_t", (P, KC_H, S))            # word_embedding[b].T chunked (f32)
    tag2t = inp("tag2t", (P, KC_H, TPC), BF16)  # tag_emb[2c:2c+2].T chunked
    wv_t = inp("wv_t", (P, KC_H, H), BF16)      # Wv.T chunked [p, hc, h']
    bv_col = inp("bv_col", (P, KC_H))           # bv chunked per-partition
    op_t = inp("op_t", (P, KC_H, H), BF16)      # out_proj_w.T chunked
    ob_col = inp("ob_col", (P, KC_H))
    ff1t_c = inp("ff1t_c", (P, TPC * KC_H, H), BF16)  # ff1_w.T rows (2 tags) chunked
    ff1b_col = inp("ff1b_col", (P, KC_H))
    ff2t = inp("ff2t", (P, KC_H, H2), BF16)     # ff2_w.T chunked
    ff2b_col = inp("ff2b_col", (P, KC_H2))
    g_col = inp("g_col", (P, KC_F))
    lwg2 = inp("lwg2", (P, KC_F, ML), BF16)     # [lin_w.T | 0pad | ones] (g folded on dev)
    lw_b = inp("lw_b", (P, KC_F, NL), BF16)     # lin_w.T (for c2)
    b_col = inp("b_col", (P, KC_F), BF16)
    lin_b = inp("lin_b", (NL, 1))
    sp_start = inp("sp_start", (P, n_span_tiles))
    sp_end = inp("sp_end", (P, n_span_tiles))
    sp_tag = inp("sp_tag", (P, n_span_tiles))
    iota_s = inp("iota_s", (P, S), F16)         # 0..S-1 on every partition
    iota_t = inp("iota_t", (P, T), F16)

    out = nc.dram_tensor("out", [NL, S], F32, kind="ExternalOutput").ap()

    with tile.TileContext(nc) as tc:
        with (
            tc.tile_pool(name="singles", bufs=1) as singles,
            tc.tile_pool(name="spans", bufs=2) as spans,
            tc.tile_pool(name="work", bufs=2) as work,
            tc.tile_pool(name="stats", bufs=1) as stats,
            tc.tile_pool(name="ps_mm", bufs=2, space="PSUM") as ps_mm,
            tc.tile_pool(name="ps_big", bufs=3, space="PSUM") as ps_big,
            tc.tile_pool(name="ps_acc", bufs=1, space="PSUM") as ps_acc,
            tc.tile_pool(name="dram", bufs=1, space="DRAM") as dram,
        ):
            # ---- constants -------------------------------------------------
            ones_col = singles.tile([P, 1], BF16)
            nc.vector.memset(ones_col, 1.0)
            ones_colf = singles.tile([P, 1], F32)
            nc.vector.memset(ones_colf, 1.0)
            eps_t = singles.tile([1, 1], F32)
            nc.vector.memset(eps_t, EPS)
            ones_row = singles.tile([1, NL], F32)
            nc.vector.memset(ones_row, 1.0)
            neg_ones = singles.tile([P, 1], BF16)
            nc.vector.memset(neg_ones, -1.0)
            scratch = singles.tile([1, 1], F32)

            # ---- DMA queue: W_eff-path loads first (they gate the AllGather)
            tag2_sb = singles.tile([P, KC_H, TPC], BF16)
            nc.sync.dma_start(out=tag2_sb, in_=tag2t)
            bv_sb = singles.tile([P, KC_H], F32)
            nc.sync.dma_start(out=bv_sb, in_=bv_col)
            ob_sb = singles.tile([P, KC_H], F32)
            nc.sync.dma_start(out=ob_sb, in_=ob_col)
            # wv/op split across the SP and ACT queues so both land early;
            # ff1 tl0 chunked so the W_eff matmuls track DMA arrivals
            wv_sb = singles.tile([P, KC_H, H], BF16)
            nc.sync.dma_start(out=wv_sb[:, :3, :], in_=wv_t[:, :3, :])
            nc.scalar.dma_start(out=wv_sb[:, 3:, :], in_=wv_t[:, 3:, :])
            op_sb = singles.tile([P, KC_H, H], BF16)
            nc.sync.dma_start(out=op_sb[:, :3, :], in_=op_t[:, :3, :])
            nc.scalar.dma_start(out=op_sb[:, 3:, :], in_=op_t[:, 3:, :])
            ff1_sb = singles.tile([P, TPC * KC_H, H], BF16)
            for kk in range(KC_H):
                nc.sync.dma_start(
                    out=ff1_sb[:, kk, :], in_=ff1t_c[:, kk, :]
                )
            nc.scalar.dma_start(
                out=ff1_sb[:, KC_H:2 * KC_H, :], in_=ff1t_c[:, KC_H:2 * KC_H, :]
            )


            iota_s_sb = singles.tile([P, S], F16)
            nc.gpsimd.dma_start(out=iota_s_sb, in_=iota_s)
            iota_t_sb = singles.tile([P, T], F16)
            nc.gpsimd.dma_start(out=iota_t_sb, in_=iota_t)
            sps_sb = singles.tile([P, n_span_tiles], F32)
            spe_sb = singles.tile([P, n_span_tiles], F32)
            spt_sb = singles.tile([P, n_span_tiles], F32)
            nc.gpsimd.dma_start(out=sps_sb, in_=sp_start)
            nc.gpsimd.dma_start(out=spe_sb, in_=sp_end)
            nc.gpsimd.dma_start(out=spt_sb, in_=sp_tag)

            ff1b_sb = singles.tile([P, KC_H], F32)
            nc.sync.dma_start(out=ff1b_sb, in_=ff1b_col)
            ff2b_sb = singles.tile([P, KC_H2], F32)
            nc.sync.dma_start(out=ff2b_sb, in_=ff2b_col)
            lwg2_in = singles.tile([P, KC_F, ML], BF16)
            nc.sync.dma_start(out=lwg2_in, in_=lwg2)
            lw_sb = singles.tile([P, KC_F, NL], BF16)
            nc.sync.dma_start(out=lw_sb, in_=lw_b)
            g_sb = singles.tile([P, KC_F], F32)
            nc.sync.dma_start(out=g_sb, in_=g_col)
            b_sb = singles.tile([P, KC_F], BF16)
            nc.sync.dma_start(out=b_sb, in_=b_col)
            linb_sb = singles.tile([NL, 1], F32)
            nc.sync.dma_start(out=linb_sb, in_=lin_b)
            we_sb = singles.tile([P, KC_H, S], F32)
            nc.sync.dma_start(out=we_sb, in_=we_t)
            ff2_sb = singles.tile([P, KC_H, H2], BF16)
            nc.sync.dma_start(out=ff2_sb, in_=ff2t)

            # ================= overlapped with the AllGather =================
            # ---- counts: masks on DVE, accumulate on PE --------------------
            counts_ps = ps_acc.tile([T, S], F32, tag="counts")
            for i in range(n_span_tiles):
                # coverage mask = (s < end) - (s < start); the subtraction is
                # folded into the PE accumulation via a negated onehot.
                lt_e = spans.tile([P, S], BF16, tag="lt_e")
                lt_s = spans.tile([P, S], BF16, tag="lt_s")
                mask = spans.tile([P, S], BF16, tag="mask")
                nc.vector.tensor_scalar(
                    out=lt_e, in0=iota_s_sb, scalar1=spe_sb[:, i:i + 1], scalar2=None,
                    op0=mybir.AluOpType.is_lt,
                )
                nc.vector.tensor_scalar(
                    out=lt_s, in0=iota_s_sb, scalar1=sps_sb[:, i:i + 1], scalar2=None,
                    op0=mybir.AluOpType.is_ge,
                )
                nc.vector.tensor_mul(out=mask, in0=lt_e, in1=lt_s)
                onehot = spans.tile([P, T], BF16, tag="onehot")
                nc.vector.tensor_scalar(
                    out=onehot, in0=iota_t_sb, scalar1=spt_sb[:, i:i + 1], scalar2=None,
                    op0=mybir.AluOpType.is_equal,
                )
                nc.tensor.matmul(
                    counts_ps, onehot, mask,
                    start=(i == 0), stop=(i == n_span_tiles - 1),
                )
            # ---- W_eff chain ----------------------------------------------
            def mmT_2xH(w_sb, rhs_chunks, bias_col, dst_sb, pfx):
                """dst[p, jc, t] = sum_h w[h, j] * rhs[h, t] + bias[j]: result
                arrives already transposed (j on partitions)."""
                for jc in range(KC_H):
                    ps = ps_mm.tile([P, TPC], F32, tag="mm", name=f"{pfx}{jc}")
                    for hc in range(KC_H):
                        nc.tensor.matmul(
                            ps,
                            w_sb[:, hc, jc * P:(jc + 1) * P],
                            rhs_chunks[hc],
                            start=(hc == 0),
                            stop=(hc == KC_H - 1),
                        )
                    nc.vector.tensor_scalar(
                        out=dst_sb[:, jc, :], in0=ps,
                        scalar1=bias_col[:, jc:jc + 1], scalar2=None,
                        op0=mybir.AluOpType.add,
                    )

            vT_sb = singles.tile([P, KC_H, TPC], BF16)
            mmT_2xH(wv_sb, [tag2_sb[:, hc, :] for hc in range(KC_H)], bv_sb,
                    vT_sb, "psv")
            vtT_sb = singles.tile([P, KC_H, TPC], BF16)
            mmT_2xH(op_sb, [vT_sb[:, hc, :] for hc in range(KC_H)], ob_sb,
                    vtT_sb, "psvt")

            # W_eff local rows: W[tl, j] = sum_h vt[tl, h] * ff1T[tl*H + h, j]
            wloc_sb = singles.tile([1, TPC * H], BF16)
            for tl in range(TPC):
                pss = [ps_mm.tile([1, HH], F32, tag="mm", name=f"ps_w{tl}_{nn}")
                       for nn in range(2)]
                for kk in range(KC_H):
                    for nn in range(2):
                        nc.tensor.matmul(
                            pss[nn],
                            vtT_sb[:, kk, tl:tl + 1],
                            ff1_sb[:, tl * KC_H + kk, nn * HH:(nn + 1) * HH],
                            start=(kk == 0),
                            stop=(kk == KC_H - 1),
                        )
                for nn in range(2):
                    nc.vector.tensor_copy(
                        out=wloc_sb[:, tl * H + nn * HH:tl * H + (nn + 1) * HH],
                        in_=pss[nn],
                    )

            # AllGather W_eff: [TPC, H] per core -> [T, H].  Bounce DMAs ride
            # the gpsimd queue (SP's FIFO is full of bulk loads).
            ag_in = dram.tile([1, TPC * H], BF16)
            ag_out = dram.tile([T, H], BF16)
            nc.gpsimd.dma_start(out=ag_in, in_=wloc_sb)
            nc.gpsimd.collective_compute(
                "AllGather",
                mybir.AluOpType.bypass,
                replica_groups=[list(range(NCORES))],
                ins=[ag_in.opt()],
                outs=[ag_out.opt()],
            )
            weff_sb = singles.tile([T, H], BF16)
            nc.sync.dma_start(out=weff_sb[:, :HH], in_=ag_out[:, :HH])
            nc.sync.dma_start(out=weff_sb[:, HH:], in_=ag_out[:, HH:])

            counts_sb = singles.tile([T, S], BF16)
            nc.vector.tensor_copy(out=counts_sb, in_=counts_ps)

            # ---- lwg prep + c1/c2 ------------------------------------------
            lwg2_sb = singles.tile([P, KC_F, ML], BF16)
            lwg2f_sb = singles.tile([P, KC_H, ML], F32)
            for fc in range(KC_F):
                nc.vector.tensor_copy(
                    out=lwg2_sb[:, fc, NL:], in_=lwg2_in[:, fc, NL:]
                )
                nc.vector.tensor_scalar_mul(
                    out=lwg2_sb[:, fc, 0:NL], in0=lwg2_in[:, fc, 0:NL],
                    scalar1=g_sb[:, fc:fc + 1],
                )
            for fc in range(KC_H):
                nc.vector.tensor_copy(
                    out=lwg2f_sb[:, fc, NL:], in_=lwg2_in[:, fc, NL:]
                )
                nc.vector.tensor_scalar_mul(
                    out=lwg2f_sb[:, fc, 0:NL], in0=lwg2_in[:, fc, 0:NL],
                    scalar1=g_sb[:, fc:fc + 1],
                )
            psc1 = ps_mm.tile([1, NL], F32, tag="mm")
            psc2 = ps_mm.tile([NL, 1], F32, tag="mm")
            for fc in range(KC_F):
                nc.tensor.matmul(
                    psc1, neg_ones, lwg2_sb[:, fc, 0:NL],
                    start=(fc == 0), stop=(fc == KC_F - 1),
                )
                nc.tensor.matmul(
                    psc2, lw_sb[:, fc, :], b_sb[:, fc:fc + 1],
                    start=(fc == 0), stop=(fc == KC_F - 1),
                )
            c1n_sb = singles.tile([1, NL], F32)
            nc.vector.tensor_copy(out=c1n_sb, in_=psc1)
            c2_sb = singles.tile([NL, 1], F32)
            nc.vector.tensor_add(out=c2_sb, in0=psc2, in1=linb_sb)

            # ---- word-embedding part of raw / sum / sumsq (fc = 0..5) ------
            pr_we = ps_acc.tile([ML, S], F32, tag="pr")
            ss_we = ps_acc.tile([1, S], F32, tag="ss")
            for fc in range(KC_H):
                nc.tensor.matmul(
                    pr_we, lwg2f_sb[:, fc, :], we_sb[:, fc, :],
                    start=(fc == 0), stop=(fc == KC_H - 1),
                )
                sq = work.tile([P, S], BF16, tag="sq")
                nc.scalar.square(out=sq, in_=we_sb[:, fc, :])
                nc.tensor.matmul(
                    ss_we, ones_col, sq,
                    start=(fc == 0), stop=(fc == KC_H - 1),
                )
            # park the word-embedding halves in SBUF (frees their psum banks
            # and keeps every accumulation group contiguous and same-dtype)
            prwe_sb = singles.tile([ML, S], F32)
            nc.vector.tensor_copy(out=prwe_sb, in_=pr_we)
            sswe_sb = singles.tile([1, S], F32)
            nc.vector.tensor_copy(out=sswe_sb, in_=ss_we)
            # prefetch the Relu table while the collective is in flight
            nc.scalar.activation(
                out=scratch, in_=eps_t,
                func=mybir.ActivationFunctionType.Relu,
            )

            # ================= post-AllGather tail ==========================
            # h1 = relu(counts.T @ W_eff + ff1_b), stored transposed [H, S]
            h1r_sb = singles.tile([P, KC_H, S], BF16)
            for kj in range(KC_H):
                ps = ps_big.tile([P, S], F32, tag="big")
                nc.tensor.matmul(
                    ps, weff_sb[:, kj * P:(kj + 1) * P], counts_sb,
                    start=True, stop=True,
                )
                if kj % 2 == 0:
                    nc.scalar.activation(
                        out=h1r_sb[:, kj, :], in_=ps,
                        func=mybir.ActivationFunctionType.Relu,
                        bias=ff1b_sb[:, kj:kj + 1], scale=1.0,
                    )
                else:
                    nc.vector.tensor_scalar(
                        out=h1r_sb[:, kj, :], in0=ps,
                        scalar1=ff1b_sb[:, kj:kj + 1], scalar2=0.0,
                        op0=mybir.AluOpType.add, op1=mybir.AluOpType.max,
                    )
            # prefetch the Sqrt table before the stats need it
            nc.scalar.activation(
                out=scratch, in_=eps_t,
                func=mybir.ActivationFunctionType.Sqrt,
            )

            # h2 = relu_h1 @ ff2.T + ff2_b, stored transposed [H2, S]
            xh2_sb = singles.tile([P, KC_H2, S], BF16)
            for mc in range(KC_H2):
                ps = ps_big.tile([P, S], F32, tag="big")
                for kj in range(KC_H):
                    nc.tensor.matmul(
                        ps,
                        ff2_sb[:, kj, mc * P:(mc + 1) * P],
                        h1r_sb[:, kj, :],
                        start=(kj == 0), stop=(kj == KC_H - 1),
                    )
                nc.vector.tensor_scalar(
                    out=xh2_sb[:, mc, :], in0=ps,
                    scalar1=ff2b_sb[:, mc:mc + 1], scalar2=None,
                    op0=mybir.AluOpType.add,
                )

            # ---- h2 part of raw / sum / sumsq (fc = 6..8) ------------------
            pr_h2 = ps_acc.tile([ML, S], F32, tag="counts")
            ss_h2 = ps_acc.tile([1, S], F32, tag="ss")
            for mc in range(KC_H2):
                fc = KC_H + mc
                nc.tensor.matmul(
                    pr_h2, lwg2_sb[:, fc, :], xh2_sb[:, mc, :],
                    start=(mc == 0), stop=(mc == KC_H2 - 1),
                )
                sq = work.tile([P, S], BF16, tag="sq")
                nc.vector.tensor_mul(
                    out=sq, in0=xh2_sb[:, mc, :], in1=xh2_sb[:, mc, :]
                )
                nc.tensor.matmul(
                    ss_h2, ones_col, sq,
                    start=(mc == 0), stop=(mc == KC_H2 - 1),
                )

            # ---- stats ------------------------------------------------------
            sum_sb = stats.tile([1, S], F32, tag="sum")
            nc.vector.tensor_add(
                out=sum_sb, in0=pr_h2[ML - 1:ML, :], in1=prwe_sb[ML - 1:ML, :]
            )
            mu_sb = stats.tile([1, S], F32, tag="mu")
            nc.vector.tensor_scalar_mul(out=mu_sb, in0=sum_sb, scalar1=1.0 / NEW_H)
            sst_sb = stats.tile([1, S], F32, tag="sst")
            nc.vector.tensor_add(out=sst_sb, in0=ss_h2, in1=sswe_sb)
            ex2_sb = stats.tile([1, S], F32, tag="ex2")
            nc.vector.tensor_scalar_mul(out=ex2_sb, in0=sst_sb, scalar1=1.0 / NEW_H)
            # raw = we part + h2 part
            a_sb = stats.tile([NL, S], F32, tag="araw")
            nc.vector.tensor_add(
                out=a_sb, in0=pr_h2[0:NL, :], in1=prwe_sb[0:NL, :]
            )
            # -c1 (x) mu as its own (clean) K=1 accumulation
            c1mu_ps = ps_big.tile([NL, S], F32, tag="big")
            nc.tensor.matmul(c1mu_ps, c1n_sb, mu_sb, start=True, stop=True)
            x1_sb = stats.tile([NL, S], F32, tag="x1")
            nc.vector.tensor_add(out=x1_sb, in0=c1mu_ps, in1=a_sb)

            mu2_sb = stats.tile([1, S], F32, tag="mu2")
            nc.vector.tensor_mul(out=mu2_sb, in0=mu_sb, in1=mu_sb)
            var_sb = stats.tile([1, S], F32, tag="var")
            nc.vector.tensor_sub(out=var_sb, in0=ex2_sb, in1=mu2_sb)
            sd_sb = stats.tile([1, S], F32, tag="sd")
            nc.scalar.activation(
                out=sd_sb, in_=var_sb, func=mybir.ActivationFunctionType.Sqrt,
                bias=eps_t, scale=1.0,
            )
            rstd_sb = stats.tile([1, S], F32, tag="rstd")
            nc.vector.reciprocal(out=rstd_sb, in_=sd_sb)

            # broadcast rstd across NL partitions via a K=1 matmul
            rb_ps = ps_big.tile([NL, S], F32, tag="big")
            nc.tensor.matmul(rb_ps, ones_row, rstd_sb, start=True, stop=True)

            # final = (raw - c1*mu) * rstd + c2
            t2_sb = stats.tile([NL, S], F32, tag="t2")
            nc.vector.tensor_mul(out=t2_sb, in0=rb_ps, in1=x1_sb)
            f_sb = stats.tile([NL, S], F32, tag="fin")
            nc.vector.tensor_scalar(
                out=f_sb, in0=t2_sb, scalar1=c2_sb, scalar2=None,
                op0=mybir.AluOpType.add,
            )
            nc.sync.dma_start(out=out, in_=f_sb)

    nc.compile()
    return nc


def _chunked(a, kc):
    """[kc*128, N...] -> [128, kc, N...] (partition-major chunk layout)."""
    return np.ascontiguousarray(
        a.reshape(kc, P, *a.shape[1:]).transpose(1, 0, *range(2, a.ndim + 1))
    )


_CACHE = {}


def kernel(**inputs) -> np.ndarray:
    bfl = ml_dtypes.bfloat16
    we = np.asarray(inputs["word_embedding"], np.float32)
    te = np.asarray(inputs["tag_embedding"], np.float32)
    ipw = np.asarray(inputs["in_proj_w"], np.float32)
    ipb = np.asarray(inputs["in_proj_b"], np.float32)
    opw = np.asarray(inputs["out_proj_w"], np.float32)
    ob_ = np.asarray(inputs["out_proj_b"], np.float32)
    f1w = np.asarray(inputs["ff1_w"], np.float32)
    f1b = np.asarray(inputs["ff1_b"], np.float32)
    f2w = np.asarray(inputs["ff2_w"], np.float32)
    f2b = np.asarray(inputs["ff2_b"], np.float32)
    lg = np.asarray(inputs["ln_g"], np.float32)
    lb = np.asarray(inputs["ln_b"], np.float32)
    lw = np.asarray(inputs["lin_w"], np.float32)
    lbias = np.asarray(inputs["lin_b"], np.float32)
    sb = np.asarray(inputs["span_batch"]).astype(np.int64)
    st = np.asarray(inputs["span_tag"]).astype(np.int64)
    ss = np.asarray(inputs["span_start"]).astype(np.int64)
    se = np.asarray(inputs["span_end"]).astype(np.int64)

    # ---- host-side sharding / layout prep -----------------------------
    counts_per_b = np.bincount(sb, minlength=B)
    n_span_tiles = max(1, int(np.ceil(counts_per_b.max() / P)))
    n_pad = n_span_tiles * P

    wv_t = _chunked(ipw[2 * H:].T.astype(bfl), KC_H)        # [P, KC_H, H]
    bv_col = np.ascontiguousarray(ipb[2 * H:].reshape(KC_H, P).T)
    op_t = _chunked(opw.T.astype(bfl), KC_H)
    ob_col = np.ascontiguousarray(ob_.reshape(KC_H, P).T)
    ff1T = f1w.T.astype(bfl)                                # [T*H, H]
    ff2t = _chunked(f2w.T.astype(bfl), KC_H)                # [P, KC_H, H2]
    ff1b_col = np.ascontiguousarray(f1b.reshape(KC_H, P).T)
    ff2b_col = np.ascontiguousarray(f2b.reshape(KC_H2, P).T)
    g_col = np.ascontiguousarray(lg.reshape(KC_F, P).T)
    b_col = np.ascontiguousarray(lb.reshape(KC_F, P).T.astype(bfl))
    lwt = lw.T.astype(bfl)                                  # [NEW_H, NL]
    lw_b = _chunked(lwt, KC_F)                              # [P, KC_F, NL]
    lwg2 = np.zeros((P, KC_F, ML), bfl)
    lwg2[:, :, ML - 1] = 1.0
    lwg2[:, :, 0:NL] = lw_b
    lin_b_col = np.ascontiguousarray(lbias.reshape(NL, 1))
    iota_s = np.ascontiguousarray(
        np.broadcast_to(np.arange(S, dtype=np.float16), (P, S))
    )
    iota_t = np.ascontiguousarray(
        np.broadcast_to(np.arange(T, dtype=np.float16), (P, T))
    )

    in_maps = []
    for c in range(NCORES):
        idx = np.where(sb == c)[0]
        n = len(idx)
        sps = np.zeros(n_pad, np.float32)
        spe = np.zeros(n_pad, np.float32)
        spt = np.zeros(n_pad, np.float32)
        sps[:n] = ss[idx]
        spe[:n] = se[idx]
        spt[:n] = st[idx]
        in_maps.append(dict(
            we_t=_chunked(np.ascontiguousarray(we[c].T), KC_H),
            tag2t=_chunked(te[c * TPC:(c + 1) * TPC].T.astype(bfl), KC_H),
            wv_t=wv_t, bv_col=bv_col, op_t=op_t, ob_col=ob_col,
            ff1t_c=_chunked(
                ff1T[c * TPC * H:(c + 1) * TPC * H], TPC * KC_H
            ),
            ff1b_col=ff1b_col, ff2t=ff2t, ff2b_col=ff2b_col,
            g_col=g_col, lwg2=lwg2, lw_b=lw_b, b_col=b_col, lin_b=lin_b_col,
            sp_start=np.ascontiguousarray(sps.reshape(n_span_tiles, P).T),
            sp_end=np.ascontiguousarray(spe.reshape(n_span_tiles, P).T),
            sp_tag=np.ascontiguousarray(spt.reshape(n_span_tiles, P).T),
            iota_s=iota_s, iota_t=iota_t,
        ))

    if n_span_tiles not in _CACHE:
        _CACHE[n_span_tiles] = build_kernel(n_span_tiles)
    nc = _CACHE[n_span_tiles]

    res = run_bass_kernel_spmd(nc, in_maps, list(range(NCORES)))
    out = np.stack([res.results[c]["out"].T for c in range(NCORES)])
    return out.astype(np.float32)


if __name__ == "__main__":
    import reference
    inp = {k: np.asarray(v) for k, v in reference.setup_inputs().items()}
    got = kernel(**inp)
    print("kernel output:", got.shape, got.dtype)


# revision 49
# speedup vs baseline: 1.0061x; 1.0061x over previous
"""Trainium2 Bass kernel for nn_Estor_concat (scatter_memory).

Math (exact reformulation of the reference):
  v_tag = (tag_emb @ Wv.T + bv) @ out_proj_w.T + out_proj_b            [T, H]
  W_eff[t, j] = sum_h v_tag[t, h] * ff1_w[j, t*H + h]                  [T, H]
  counts[t, b, s] = #spans(tag=t, batch=b) covering s
                  = sum_n onehot_t[n] * ((s < end_n) - (s < start_n))   (PE matmul)
  h1 = counts_b.T @ W_eff + ff1_b ; h2 = relu(h1) @ ff2_w.T + ff2_b
  x = [word_emb_b | h2]; LayerNorm folded into the output projection:
  out = (x @ (lin_w.T * g) - mu * c1) * rstd + (lin_w @ b + lin_b)

Sharding: data-parallel over batch (8 cores, 1 batch each); the W_eff
computation is sharded over tags (2 tags/core) with one AllGather. The
schedule front-loads the W_eff chain so the AllGather (~15us launch
latency) overlaps counts, the word-embedding half of the output/stats
accumulation, and all remaining loads.
"""

import ml_dtypes
import numpy as np

import concourse.bacc as bacc
import concourse.bass as bass
import concourse.mybir as mybir
import concourse.tile as tile
from concourse.bass_utils import run_bass_kernel_spmd

T, B, S, H = 16, 8, 512, 768
H2 = 384
NEW_H = H + H2          # 1152
NL = 33                 # num labels
EPS = 1e-12
NCORES = 8
TPC = T // NCORES       # tags per core = 2
KC_H = H // 128         # 6 chunks of the hidden dim
KC_H2 = H2 // 128       # 3
KC_F = NEW_H // 128     # 9
P = 128
HH = H // 2             # 384 (psum-bank-sized half of H)
ML = 65                 # raw-matmul lhsT cols: [sum | 31 pad | 33 labels]

F32 = mybir.dt.float32
BF16 = mybir.dt.bfloat16
F16 = mybir.dt.float16


def build_kernel(n_span_tiles: int):
    nc = bacc.Bacc(
        "TRN2",
        target_bir_lowering=False,
        debug=False,
        enable_asserts=True,
        num_devices=NCORES,
    )

    def inp(name, shape, dtype=F32):
        return nc.dram_tensor(name, list(shape), dtype, kind="ExternalInput").ap()

    # per-core inputs (host pre-sharded / pre-transposed / pre-chunked)
    we_t = inp("we_t", (P, KC_H, S))            # word_embedding[b].T chunked (f32)
    tag2t = inp("tag2t", (P, KC_H, TPC), BF16)  # tag_emb[2c:2c+2].T chunked
    wv_t = inp("wv_t", (P, KC_H, H), BF16)      # Wv.T chunked [p, hc, h']
    bv_col = inp("bv_col", (P, KC_H))           # bv chunked per-partition
    op_t = inp("op_t", (P, KC_H, H), BF16)      # out_proj_w.T chunked
    ob_col = inp("ob_col", (P, KC_H))
    ff1t_c = inp("ff1t_c", (P, TPC * KC_H, H), BF16)  # ff1_w.T rows (2 tags) chunked
    ff1b_col = inp("ff1b_col", (P, KC_H))
    ff2t = inp("ff2t", (P, KC_H, H2), BF16)     # ff2_w.T chunked
    ff2b_col = inp("ff2b_col", (P, KC_H2))
    g_col = inp("g_col", (P, KC_F))
    lwg2 = inp("lwg2", (P, KC_F, ML), BF16)     # [lin_w.T | 0pad | ones] (g folded on dev)
    lw_b = inp("lw_b", (P, KC_F, NL), BF16)     # lin_w.T (for c2)
    b_col = inp("b_col", (P, KC_F), BF16)
    lin_b = inp("lin_b", (NL, 1))
    sp_start = inp("sp_start", (P, n_span_tiles))
    sp_end = inp("sp_end", (P, n_span_tiles))
    sp_tag = inp("sp_tag", (P, n_span_tiles))
    iota_s = inp("iota_s", (P, S), F16)         # 0..S-1 on every partition
    iota_t = inp("iota_t", (P, T), F16)

    out = nc.dram_tensor("out", [NL, S], F32, kind="ExternalOutput").ap()

    with tile.TileContext(nc) as tc:
        with (
            tc.tile_pool(name="singles", bufs=1) as singles,
            tc.tile_pool(name="spans", bufs=3) as spans,
            tc.tile_pool(name="work", bufs=3) as work,
            tc.tile_pool(name="stats", bufs=1) as stats,
            tc.tile_pool(name="ps_mm", bufs=3, space="PSUM") as ps_mm,
            tc.tile_pool(name="ps_big", bufs=2, space="PSUM") as ps_big,
            tc.tile_pool(name="ps_acc", bufs=1, space="PSUM") as ps_acc,
            tc.tile_pool(name="dram", bufs=1, space="DRAM") as dram,
        ):
            # ---- constants -------------------------------------------------
            ones_col = singles.tile([P, 1], BF16)
            nc.vector.memset(ones_col, 1.0)
            ones_colf = singles.tile([P, 1], F32)
            nc.vector.memset(ones_colf, 1.0)
            eps_t = singles.tile([1, 1], F32)
            nc.vector.memset(eps_t, EPS)
            ones_row = singles.tile([1, NL], F32)
            nc.vector.memset(ones_row, 1.0)
            neg_ones = singles.tile([P, 1], BF16)
            nc.vector.memset(neg_ones, -1.0)
            scratch = singles.tile([1, 1], F32)

            # ---- DMA queue: W_eff-path loads first (they gate the AllGather)
            tag2_sb = singles.tile([P, KC_H, TPC], BF16)
            nc.sync.dma_start(out=tag2_sb, in_=tag2t)
            bv_sb = singles.tile([P, KC_H], F32)
            nc.sync.dma_start(out=bv_sb, in_=bv_col)
            ob_sb = singles.tile([P, KC_H], F32)
            nc.sync.dma_start(out=ob_sb, in_=ob_col)
            # wv/op split across the SP and ACT queues so both land early;
            # ff1 tl0 chunked so the W_eff matmuls track DMA arrivals
            wv_sb = singles.tile([P, KC_H, H], BF16)
            nc.sync.dma_start(out=wv_sb[:, :3, :], in_=wv_t[:, :3, :])
            nc.scalar.dma_start(out=wv_sb[:, 3:, :], in_=wv_t[:, 3:, :])
            op_sb = singles.tile([P, KC_H, H], BF16)
            nc.sync.dma_start(out=op_sb[:, :3, :], in_=op_t[:, :3, :])
            nc.scalar.dma_start(out=op_sb[:, 3:, :], in_=op_t[:, 3:, :])
            ff1_sb = singles.tile([P, TPC * KC_H, H], BF16)
            for kk in range(KC_H):
                nc.sync.dma_start(
                    out=ff1_sb[:, kk, :], in_=ff1t_c[:, kk, :]
                )
            nc.scalar.dma_start(
                out=ff1_sb[:, KC_H:2 * KC_H, :], in_=ff1t_c[:, KC_H:2 * KC_H, :]
            )


            iota_s_sb = singles.tile([P, S], F16)
            nc.gpsimd.dma_start(out=iota_s_sb, in_=iota_s)
            iota_t_sb = singles.tile([P, T], F16)
            nc.gpsimd.dma_start(out=iota_t_sb, in_=iota_t)
            sps_sb = singles.tile([P, n_span_tiles], F32)
            spe_sb = singles.tile([P, n_span_tiles], F32)
            spt_sb = singles.tile([P, n_span_tiles], F32)
            nc.gpsimd.dma_start(out=sps_sb, in_=sp_start)
            nc.gpsimd.dma_start(out=spe_sb, in_=sp_end)
            nc.gpsimd.dma_start(out=spt_sb, in_=sp_tag)

            ff1b_sb = singles.tile([P, KC_H], F32)
            nc.sync.dma_start(out=ff1b_sb, in_=ff1b_col)
            ff2b_sb = singles.tile([P, KC_H2], F32)
            nc.sync.dma_start(out=ff2b_sb, in_=ff2b_col)
            lwg2_in = singles.tile([P, KC_F, ML], BF16)
            nc.sync.dma_start(out=lwg2_in, in_=lwg2)
            lw_sb = singles.tile([P, KC_F, NL], BF16)
            nc.sync.dma_start(out=lw_sb, in_=lw_b)
            g_sb = singles.tile([P, KC_F], F32)
            nc.sync.dma_start(out=g_sb, in_=g_col)
            b_sb = singles.tile([P, KC_F], BF16)
            nc.sync.dma_start(out=b_sb, in_=b_col)
            linb_sb = singles.tile([NL, 1], F32)
            nc.sync.dma_start(out=linb_sb, in_=lin_b)
            we_sb = singles.tile([P, KC_H, S], F32)
            nc.sync.dma_start(out=we_sb, in_=we_t)
            ff2_sb = singles.tile([P, KC_H, H2], BF16)
            nc.sync.dma_start(out=ff2_sb, in_=ff2t)

            # ================= overlapped with the AllGather =================
            # ---- counts: masks on DVE, accumulate on PE --------------------
            counts_ps = ps_acc.tile([T, S], F32, tag="counts")
            for i in range(n_span_tiles):
                # coverage mask = (s < end) - (s < start); the subtraction is
                # folded into the PE accumulation via a negated onehot.
                lt_e = spans.tile([P, S], BF16, tag="lt_e")
                lt_s = spans.tile([P, S], BF16, tag="lt_s")
                mask = spans.tile([P, S], BF16, tag="mask")
                nc.vector.tensor_scalar(
                    out=lt_e, in0=iota_s_sb, scalar1=spe_sb[:, i:i + 1], scalar2=None,
                    op0=mybir.AluOpType.is_lt,
                )
                nc.vector.tensor_scalar(
                    out=lt_s, in0=iota_s_sb, scalar1=sps_sb[:, i:i + 1], scalar2=None,
                    op0=mybir.AluOpType.is_ge,
                )
                nc.vector.tensor_mul(out=mask, in0=lt_e, in1=lt_s)
                onehot = spans.tile([P, T], BF16, tag="onehot")
                nc.vector.tensor_scalar(
                    out=onehot, in0=iota_t_sb, scalar1=spt_sb[:, i:i + 1], scalar2=None,
                    op0=mybir.AluOpType.is_equal,
                )
                nc.tensor.matmul(
                    counts_ps, onehot, mask,
                    start=(i == 0), stop=(i == n_span_tiles - 1),
                )
            # ---- W_eff chain ----------------------------------------------
            def mmT_2xH(w_sb, rhs_chunks, bias_col, dst_sb, pfx):
                """dst[p, jc, t] = sum_h w[h, j] * rhs[h, t] + bias[j]: result
                arrives already transposed (j on partitions)."""
                for jc in range(KC_H):
                    ps = ps_mm.tile([P, TPC], F32, tag="mm", name=f"{pfx}{jc}")
                    for hc in range(KC_H):
                        nc.tensor.matmul(
                            ps,
                            w_sb[:, hc, jc * P:(jc + 1) * P],
                            rhs_chunks[hc],
                            start=(hc == 0),
                            stop=(hc == KC_H - 1),
                        )
                    nc.vector.tensor_scalar(
                        out=dst_sb[:, jc, :], in0=ps,
                        scalar1=bias_col[:, jc:jc + 1], scalar2=None,
                        op0=mybir.AluOpType.add,
                    )

            vT_sb = singles.tile([P, KC_H, TPC], BF16)
            mmT_2xH(wv_sb, [tag2_sb[:, hc, :] for hc in range(KC_H)], bv_sb,
                    vT_sb, "psv")
            vtT_sb = singles.tile([P, KC_H, TPC], BF16)
            mmT_2xH(op_sb, [vT_sb[:, hc, :] for hc in range(KC_H)], ob_sb,
                    vtT_sb, "psvt")

            # W_eff local rows: W[tl, j] = sum_h vt[tl, h] * ff1T[tl*H + h, j]
            wloc_sb = singles.tile([1, TPC * H], BF16)
            for tl in range(TPC):
                pss = [ps_mm.tile([1, HH], F32, tag="mm", name=f"ps_w{tl}_{nn}")
                       for nn in range(2)]
                for kk in range(KC_H):
                    for nn in range(2):
                        nc.tensor.matmul(
                            pss[nn],
                            vtT_sb[:, kk, tl:tl + 1],
                            ff1_sb[:, tl * KC_H + kk, nn * HH:(nn + 1) * HH],
                            start=(kk == 0),
                            stop=(kk == KC_H - 1),
                        )
                for nn in range(2):
                    nc.vector.tensor_copy(
                        out=wloc_sb[:, tl * H + nn * HH:tl * H + (nn + 1) * HH],
                        in_=pss[nn],
                    )

            # AllGather W_eff: [TPC, H] per core -> [T, H].  Bounce DMAs ride
            # the gpsimd queue (SP's FIFO is full of bulk loads).
            ag_in = dram.tile([1, TPC * H], BF16)
            ag_out = dram.tile([T, H], BF16)
            nc.gpsimd.dma_start(out=ag_in, in_=wloc_sb)
            nc.gpsimd.collective_compute(
                "AllGather",
                mybir.AluOpType.bypass,
                replica_groups=[list(range(NCORES))],
                ins=[ag_in.opt()],
                outs=[ag_out.opt()],
            )
            weff_sb = singles.tile([T, H], BF16)
            nc.sync.dma_start(out=weff_sb[:, :HH], in_=ag_out[:, :HH])
            nc.sync.dma_start(out=weff_sb[:, HH:], in_=ag_out[:, HH:])

            counts_sb = singles.tile([T, S], BF16)
            nc.vector.tensor_copy(out=counts_sb, in_=counts_ps)

            # ---- lwg prep + c1/c2 ------------------------------------------
            lwg2_sb = singles.tile([P, KC_F, ML], BF16)
            lwg2f_sb = singles.tile([P, KC_H, ML], F32)
            for fc in range(KC_F):
                nc.vector.tensor_copy(
                    out=lwg2_sb[:, fc, NL:], in_=lwg2_in[:, fc, NL:]
                )
                nc.vector.tensor_scalar_mul(
                    out=lwg2_sb[:, fc, 0:NL], in0=lwg2_in[:, fc, 0:NL],
                    scalar1=g_sb[:, fc:fc + 1],
                )
            for fc in range(KC_H):
                nc.vector.tensor_copy(
                    out=lwg2f_sb[:, fc, NL:], in_=lwg2_in[:, fc, NL:]
                )
                nc.vector.tensor_scalar_mul(
                    out=lwg2f_sb[:, fc, 0:NL], in0=lwg2_in[:, fc, 0:NL],
                    scalar1=g_sb[:, fc:fc + 1],
                )
            psc1 = ps_mm.tile([1, NL], F32, tag="mm")
            psc2 = ps_mm.tile([NL, 1], F32, tag="mm")
            for fc in range(KC_F):
                nc.tensor.matmul(
                    psc1, neg_ones, lwg2_sb[:, fc, 0:NL],
                    start=(fc == 0), stop=(fc == KC_F - 1),
                )
                nc.tensor.matmul(
                    psc2, lw_sb[:, fc, :], b_sb[:, fc:fc + 1],
                    start=(fc == 0), stop=(fc == KC_F - 1),
                )
            c1n_sb = singles.tile([1, NL], F32)
            nc.vector.tensor_copy(out=c1n_sb, in_=psc1)
            c2_sb = singles.tile([NL, 1], F32)
            nc.vector.tensor_add(out=c2_sb, in0=psc2, in1=linb_sb)

            # ---- word-embedding part of raw / sum / sumsq (fc = 0..5) ------
            pr_we = ps_acc.tile([ML, S], F32, tag="pr")
            ss_we = ps_acc.tile([1, S], F32, tag="ss")
            for fc in range(KC_H):
                nc.tensor.matmul(
                    pr_we, lwg2f_sb[:, fc, :], we_sb[:, fc, :],
                    start=(fc == 0), stop=(fc == KC_H - 1),
                )
                sq = work.tile([P, S], BF16, tag="sq")
                nc.scalar.square(out=sq, in_=we_sb[:, fc, :])
                nc.tensor.matmul(
                    ss_we, ones_col, sq,
                    start=(fc == 0), stop=(fc == KC_H - 1),
                )
            # park the word-embedding halves in SBUF (frees their psum banks
            # and keeps every accumulation group contiguous and same-dtype)
            prwe_sb = singles.tile([ML, S], F32)
            nc.vector.tensor_copy(out=prwe_sb, in_=pr_we)
            sswe_sb = singles.tile([1, S], F32)
            nc.vector.tensor_copy(out=sswe_sb, in_=ss_we)
            # prefetch the Relu table while the collective is in flight
            nc.scalar.activation(
                out=scratch, in_=eps_t,
                func=mybir.ActivationFunctionType.Relu,
            )

            # ================= post-AllGather tail ==========================
            # h1 = relu(counts.T @ W_eff + ff1_b), stored transposed [H, S]
            h1r_sb = singles.tile([P, KC_H, S], BF16)
            for kj in range(KC_H):
                ps = ps_big.tile([P, S], F32, tag="big")
                nc.tensor.matmul(
                    ps, weff_sb[:, kj * P:(kj + 1) * P], counts_sb,
                    start=True, stop=True,
                )
                if kj % 2 == 0:
                    nc.scalar.activation(
                        out=h1r_sb[:, kj, :], in_=ps,
                        func=mybir.ActivationFunctionType.Relu,
                        bias=ff1b_sb[:, kj:kj + 1], scale=1.0,
                    )
                else:
                    nc.vector.tensor_scalar(
                        out=h1r_sb[:, kj, :], in0=ps,
                        scalar1=ff1b_sb[:, kj:kj + 1], scalar2=0.0,
                        op0=mybir.AluOpType.add, op1=mybir.AluOpType.max,
                    )
            # prefetch the Sqrt table before the stats need it
            nc.scalar.activation(
                out=scratch, in_=eps_t,
                func=mybir.ActivationFunctionType.Sqrt,
            )

            # h2 = relu_h1 @ ff2.T + ff2_b, stored transposed [H2, S]
            xh2_sb = singles.tile([P, KC_H2, S], BF16)
            for mc in range(KC_H2):
                ps = ps_big.tile([P, S], F32, tag="big")
                for kj in range(KC_H):
                    nc.tensor.matmul(
                        ps,
                        ff2_sb[:, kj, mc * P:(mc + 1) * P],
                        h1r_sb[:, kj, :],
                        start=(kj == 0), stop=(kj == KC_H - 1),
                    )
                nc.vector.tensor_scalar(
                    out=xh2_sb[:, mc, :], in0=ps,
                    scalar1=ff2b_sb[:, mc:mc + 1], scalar2=None,
                    op0=mybir.AluOpType.add,
                )

            # ---- h2 part of raw / sum / sumsq (fc = 6..8) ------------------
            pr_h2 = ps_acc.tile([ML, S], F32, tag="counts")
            ss_h2 = ps_acc.tile([1, S], F32, tag="ss")
            for mc in range(KC_H2):
                fc = KC_H + mc
                nc.tensor.matmul(
                    pr_h2, lwg2_sb[:, fc, :], xh2_sb[:, mc, :],
                    start=(mc == 0), stop=(mc == KC_H2 - 1),
                )
                sq = work.tile([P, S], BF16, tag="sq")
                nc.vector.tensor_mul(
                    out=sq, in0=xh2_sb[:, mc, :], in1=xh2_sb[:, mc, :]
                )
                nc.tensor.matmul(
                    ss_h2, ones_col, sq,
                    start=(mc == 0), stop=(mc == KC_H2 - 1),
                )

            # ---- stats ------------------------------------------------------
            sum_sb = stats.tile([1, S], F32, tag="sum")
            nc.vector.tensor_add(
                out=sum_sb, in0=pr_h2[ML - 1:ML, :], in1=prwe_sb[ML - 1:ML, :]
            )
            mu_sb = stats.tile([1, S], F32, tag="mu")
            nc.vector.tensor_scalar_mul(out=mu_sb, in0=sum_sb, scalar1=1.0 / NEW_H)
            sst_sb = stats.tile([1, S], F32, tag="sst")
            nc.vector.tensor_add(out=sst_sb, in0=ss_h2, in1=sswe_sb)
            ex2_sb = stats.tile([1, S], F32, tag="ex2")
            nc.vector.tensor_scalar_mul(out=ex2_sb, in0=sst_sb, scalar1=1.0 / NEW_H)
            # raw = we part + h2 part
            a_sb = stats.tile([NL, S], F32, tag="araw")
            nc.vector.tensor_add(
                out=a_sb, in0=pr_h2[0:NL, :], in1=prwe_sb[0:NL, :]
            )
            # -c1 (x) mu as its own (clean) K=1 accumulation
            c1mu_ps = ps_big.tile([NL, S], F32, tag="big")
            nc.tensor.matmul(c1mu_ps, c1n_sb, mu_sb, start=True, stop=True)
            x1_sb = stats.tile([NL, S], F32, tag="x1")
            nc.vector.tensor_add(out=x1_sb, in0=c1mu_ps, in1=a_sb)

            mu2_sb = stats.tile([1, S], F32, tag="mu2")
            nc.vector.tensor_mul(out=mu2_sb, in0=mu_sb, in1=mu_sb)
            var_sb = stats.tile([1, S], F32, tag="var")
            nc.vector.tensor_sub(out=var_sb, in0=ex2_sb, in1=mu2_sb)
            sd_sb = stats.tile([1, S], F32, tag="sd")
            nc.scalar.activation(
                out=sd_sb, in_=var_sb, func=mybir.ActivationFunctionType.Sqrt,
                bias=eps_t, scale=1.0,
            )
            rstd_sb = stats.tile([1, S], F32, tag="rstd")
            nc.vector.reciprocal(out=rstd_sb, in_=sd_sb)

            # broadcast rstd across NL partitions via a K=1 matmul
            rb_ps = ps_big.tile([NL, S], F32, tag="big")
            nc.tensor.matmul(rb_ps, ones_row, rstd_sb, start=True, stop=True)

            # final = (raw - c1*mu) * rstd + c2
            t2_sb = stats.tile([NL, S], F32, tag="t2")
            nc.vector.tensor_mul(out=t2_sb, in0=rb_ps, in1=x1_sb)
            f_sb = stats.tile([NL, S], F32, tag="fin")
            nc.vector.tensor_scalar(
                out=f_sb, in0=t2_sb, scalar1=c2_sb, scalar2=None,
                op0=mybir.AluOpType.add,
            )
            nc.sync.dma_start(out=out, in_=f_sb)

    nc.compile()
    return nc


def _chunked(a, kc):
    """[kc*128, N...] -> [128, kc, N...] (partition-major chunk layout)."""
    return np.ascontiguousarray(
        a.reshape(kc, P, *a.shape[1:]).transpose(1, 0, *range(2, a.ndim + 1))
    )


_CACHE = {}


def kernel(**inputs) -> np.ndarray:
    bfl = ml_dtypes.bfloat16
    we = np.asarray(inputs["word_embedding"], np.float32)
    te = np.asarray(inputs["tag_embedding"], np.float32)
    ipw = np.asarray(inputs["in_proj_w"], np.float32)
    ipb = np.asarray(inputs["in_proj_b"], np.float32)
    opw = np.asarray(inputs["out_proj_w"], np.float32)
    ob_ = np.asarray(inputs["out_proj_b"], np.float32)
    f1w = np.asarray(inputs["ff1_w"], np.float32)
    f1b = np.asarray(inputs["ff1_b"], np.float32)
    f2w = np.asarray(inputs["ff2_w"], np.float32)
    f2b = np.asarray(inputs["ff2_b"], np.float32)
    lg = np.asarray(inputs["ln_g"], np.float32)
    lb = np.asarray(inputs["ln_b"], np.float32)
    lw = np.asarray(inputs["lin_w"], np.float32)
    lbias = np.asarray(inputs["lin_b"], np.float32)
    sb = np.asarray(inputs["span_batch"]).astype(np.int64)
    st = np.asarray(inputs["span_tag"]).astype(np.int64)
    ss = np.asarray(inputs["span_start"]).astype(np.int64)
    se = np.asarray(inputs["span_end"]).astype(np.int64)

    # ---- host-side sharding / layout prep -----------------------------
    counts_per_b = np.bincount(sb, minlength=B)
    n_span_tiles = max(1, int(np.ceil(counts_per_b.max() / P)))
    n_pad = n_span_tiles * P

    wv_t = _chunked(ipw[2 * H:].T.astype(bfl), KC_H)        # [P, KC_H, H]
    bv_col = np.ascontiguousarray(ipb[2 * H:].reshape(KC_H, P).T)
    op_t = _chunked(opw.T.astype(bfl), KC_H)
    ob_col = np.ascontiguousarray(ob_.reshape(KC_H, P).T)
    ff1T = f1w.T.astype(bfl)                                # [T*H, H]
    ff2t = _chunked(f2w.T.astype(bfl), KC_H)                # [P, KC_H, H2]
    ff1b_col = np.ascontiguousarray(f1b.reshape(KC_H, P).T)
    ff2b_col = np.ascontiguousarray(f2b.reshape(KC_H2, P).T)
    g_col = np.ascontiguousarray(lg.reshape(KC_F, P).T)
    b_col = np.ascontiguousarray(lb.reshape(KC_F, P).T.astype(bfl))
    lwt = lw.T.astype(bfl)                                  # [NEW_H, NL]
    lw_b = _chunked(lwt, KC_F)                              # [P, KC_F, NL]
    lwg2 = np.zeros((P, KC_F, ML), bfl)
    lwg2[:, :, ML - 1] = 1.0
    lwg2[:, :, 0:NL] = lw_b
    lin_b_col = np.ascontiguousarray(lbias.reshape(NL, 1))
    iota_s = np.ascontiguousarray(
        np.broadcast_to(np.arange(S, dtype=np.float16), (P, S))
    )
    iota_t = np.ascontiguousarray(
        np.broadcast_to(np.arange(T, dtype=np.float16), (P, T))
    )

    in_maps = []
    for c in range(NCORES):
        idx = np.where(sb == c)[0]
        n = len(idx)
        sps = np.zeros(n_pad, np.float32)
        spe = np.zeros(n_pad, np.float32)
        spt = np.zeros(n_pad, np.float32)
        sps[:n] = ss[idx]
        spe[:n] = se[idx]
        spt[:n] = st[idx]
        in_maps.append(dict(
            we_t=_chunked(np.ascontiguousarray(we[c].T), KC_H),
            tag2t=_chunked(te[c * TPC:(c + 1) * TPC].T.astype(bfl), KC_H),
            wv_t=wv_t, bv_col=bv_col, op_t=op_t, ob_col=ob_col,
            ff1t_c=_chunked(
                ff1T[c * TPC * H:(c + 1) * TPC * H], TPC * KC_H
            ),
            ff1b_col=ff1b_col, ff2t=ff2t, ff2b_col=ff2b_col,
            g_col=g_col, lwg2=lwg2, lw_b=lw_b, b_col=b_col, lin_b=lin_b_col,
            sp_start=np.ascontiguousarray(sps.reshape(n_span_tiles, P).T),
            sp_end=np.ascontiguousarray(spe.reshape(n_span_tiles, P).T),
            sp_tag=np.ascontiguousarray(spt.reshape(n_span_tiles, P).T),
            iota_s=iota_s, iota_t=iota_t,
        ))

    if n_span_tiles not in _CACHE:
        _CACHE[n_span_tiles] = build_kernel(n_span_tiles)
    nc = _CACHE[n_span_tiles]

    res = run_bass_kernel_spmd(nc, in_maps, list(range(NCORES)))
    out = np.stack([res.results[c]["out"].T for c in range(NCORES)])
    return out.astype(np.float32)


if __name__ == "__main__":
    import reference
    inp = {k: np.asarray(v) for k, v in reference.setup_inputs().items()}
    got = kernel(**inp)
    print("kernel output:", got.shape, got.dtype)
